# revision 23
# baseline (speedup 1.0000x reference)
"""CapsuleLayer (dynamic routing) Trainium2 kernel, SPMD over 8 NeuronCores.

Sharding: input-capsule axis (IN_CAPS=512 -> 64 per core). W and u_hat are
i-sharded; the bij,bijd->bjd contraction is completed with AllReduces of
bf16 s-partials once per routing iteration.

Per-core layout (i_local = i2*32 + i1, i2 in {0,1}):
  u_hat SBUF [p=(i2*64+b), (d, i1, j)] bf16 -- 128 partitions x 16384
  b/c logits [p, (i1, j)], s partials [p, (d, j)].

v2 structure (over the 245us baseline):
- No warmup AllReduce. Instead, the r=0 s-partial (c uniform -> s0 =
  (1/16) sum_i u_hat, the 1/16 folded into the squash scale) is built
  incrementally DURING phase 1: each 4-i group is folded 4->1 as its
  PSUM cast lands, staged into gfs[128, 8, (d,j)], and tree-folded 8->1
  right after the last cast. Its AllReduce triggers at ~46us, so the AR
  starts the moment the NRT collective-init barrier (48-145us, run
  variance) ends, instead of queueing behind a warmup AR + DVE fold
  (~36us of serialized warmup+AR0 on the baseline critical path).
- Every s AllReduce is split into two d-halves (64 KB bf16 each) on the
  single CC stream. Downstream work is d-decomposable: the agreement
  update b_log += sum_d out*u_hat = scale[b,j] * sum_d s*u_hat (squash
  scale applied AFTER the d-fold, algebraically identical), so the
  bu-mul+fold for half A runs while half B's AR is still in flight.
  Per-iteration AR exposure drops from ~12.5us + full serial DVE to
  mostly-hidden.
- cc buffers are dedicated (bufs = exact tile count) so no DRAM-pool
  aliasing creates false WAR semaphore deps on the AR triggers.
- sqrt(ss+eps) = exp(0.5*ln(ss+eps)) with activation tables pinned to
  the one set containing Ln+Exp+Copy (no per-iteration table reloads).

Measured notes (from traces): DVE tensor_tensor bf16 runs 2x only with
step-1 innermost and full-width ops; broadcast over a non-innermost dim
keeps 2x. AR latency is fixed ~11-13us nearly independent of payload
(128B vs 128KB), so splitting costs stream time but buys overlap. The
NRT barrier (collective stack init) ends 48-145us into the kernel and
gates the first AR; nothing in-kernel controls it. GPSIMD ~1.9ns/elem
flat, ~4.7 sliced/broadcast. remote_dma hangs this runtime (axon
fake_nrt), so the collective stack is unavoidable.

Phase 1 (per i): u_hat_i[b, dj] = xT_i.T @ W_i on the PE (K=128, M=64,
N=512), all in bf16, W streamed in 1 MB pair-blocked chunks.
"""

import numpy as np

N_CORES = 8
B = 64
IN_CAPS = 512
IN_DIM = 128
N_CAPS = 16
OUT_DIM = 32
I_LOC = IN_CAPS // N_CORES          # 64 input capsules per core
I1 = 32                             # i_local = i2*32 + i1
DH = OUT_DIM // 2                   # 16, d-half for split ARs
EPS = 1e-7
GRP = 4                             # i's per PSUM group
NGRP = I_LOC // GRP                 # 16
NPAIR = NGRP // 2                   # 8 (two groups per W DMA)

# Toggled by test.py for profiling runs.
TRACE = False
TRACE_DIR = None

_cache = {}


def _emit(tc, xT, wT, out, num_routing):
    from contextlib import ExitStack

    from concourse import mybir

    nc = tc.nc
    f32 = mybir.dt.float32
    bf16 = mybir.dt.bfloat16
    R = num_routing
    ctx = ExitStack()
    singles = ctx.enter_context(tc.tile_pool(name="singles", bufs=1))
    # all 8 W pairs resident: W DMAs never wait on a slot's prior reader,
    # so both DMA queues can post their whole half of the stream up front
    wpool = ctx.enter_context(tc.tile_pool(name="wpool", bufs=NPAIR))
    pspool = ctx.enter_context(tc.tile_pool(name="pspool", bufs=2, space="PSUM"))
    small = ctx.enter_context(tc.tile_pool(name="small", bufs=2))
    gpool = ctx.enter_context(tc.tile_pool(name="gpool", bufs=2))
    # one slot per cc buffer: zero reuse -> no false WAR deps on triggers
    ccpool = ctx.enter_context(tc.tile_pool(name="ccpool", bufs=4 * R, space="DRAM"))

    # All ARs are d-split 64KB halves: the first-AR premium scales with
    # payload (64KB first AR ~15us vs 128KB first AR ~30us), and the
    # split lets bu-mul(half A) start while half B's AR is in flight.
    cc_in = [[None, None] for _ in range(R)]
    cc_out = [[None, None] for _ in range(R)]
    for r in range(R):
        for h in (0, 1):
            cc_in[r][h] = ccpool.tile(
                [2, B, DH, N_CAPS], bf16, name=f"ccin{r}{h}"
            )
            cc_out[r][h] = ccpool.tile(
                [2, B, DH, N_CAPS], bf16, name=f"ccout{r}{h}"
            )

    def allreduce(r, h):
        nc.gpsimd.collective_compute(
            "AllReduce",
            mybir.AluOpType.add,
            replica_groups=[list(range(N_CORES))],
            ins=[cc_in[r][h].opt()],
            outs=[cc_out[r][h].opt()],
        )

    # ---- phase 1: u_hat = einsum over k, per local capsule i; the r=0
    # s-partial sum_i u_hat accumulates alongside ----
    xsb = singles.tile([IN_DIM, I_LOC, B], bf16)         # [k, i, b]
    # free order (i1, d, j): the PSUM->SBUF cast is a straight contiguous
    # copy (PSUM groups are [b, i-in-group, d, j]), and every routing op
    # stays in the DVE bf16 2x mode: smul broadcasts c over middle d, bu
    # broadcasts s over outermost i1, folds slice outermost i1 / middle d
    u_hat = singles.tile([128, I1, OUT_DIM, N_CAPS], bf16)  # [(i2,b), i1, d, j]
    gfs = singles.tile([128, NPAIR, OUT_DIM, N_CAPS], bf16)  # group partials
    eps_t = singles.tile([128, 1], f32)
    nc.vector.memset(eps_t[:], EPS)

    XCH = I_LOC // 4
    for p in range(NPAIR):
        # split the 8MB W stream across BOTH HWDGE queues (sync+scalar):
        # one queue tops out ~220GB/s (4.6us/pair); two together reach the
        # ~358GB/s HBM limit. x chunks interleave so the first matmul's
        # operands land as early as possible.
        dq = nc.sync if p % 2 == 0 else nc.scalar
        if p < 4:
            dq.dma_start(
                xsb[:, p * XCH:(p + 1) * XCH, :],
                xT[:, p * XCH:(p + 1) * XCH, :],
            )
        wtile = wpool.tile([IN_DIM, 2, GRP, OUT_DIM, N_CAPS], bf16)
        dq.dma_start(wtile[:], wT[p])
        for g2 in range(2):
            g = 2 * p + g2
            i2 = (g * GRP) // I1
            i1g = (g * GRP) % I1
            gh = (g % NPAIR)  # staging slot within this i2-half
            ps = pspool.tile([128, GRP, OUT_DIM, N_CAPS], f32)
            for t in range(GRP):
                i = g * GRP + t
                nc.tensor.matmul(
                    ps[i2 * B:(i2 + 1) * B, t], xsb[:, i, :], wtile[:, g2, t],
                    start=True, stop=True,
                )
            # copy+cast PSUM f32 -> SBUF bf16: the ~2.3us f32-src casts are
            # the phase-1 engine floor (36.8us total), split 12 ACT / 4 DVE
            # so ACT-casts and DVE-casts+folds finish together (GPSIMD
            # cannot read PSUM)
            if g % 4 == 3:
                nc.vector.tensor_copy(
                    out=u_hat[i2 * B:(i2 + 1) * B, i1g:i1g + GRP],
                    in_=ps[i2 * B:(i2 + 1) * B],
                )
            else:
                nc.scalar.copy(
                    out=u_hat[i2 * B:(i2 + 1) * B, i1g:i1g + GRP],
                    in_=ps[i2 * B:(i2 + 1) * B],
                )
            # incremental r0 fold: this group's 4 i's -> gfs slot
            ug = u_hat[i2 * B:(i2 + 1) * B, i1g:i1g + GRP]
            gtmp = gpool.tile([B, 2, OUT_DIM, N_CAPS], bf16, name="gtmp")
            nc.vector.tensor_add(gtmp[:], ug[:, 0:2], ug[:, 2:4])
            nc.vector.tensor_add(
                gfs[i2 * B:(i2 + 1) * B, gh], gtmp[:, 0], gtmp[:, 1]
            )
    # tree-fold the 8 group partials per half -> acc0 = sum_i u_hat
    nc.vector.tensor_add(gfs[:, :4], gfs[:, :4], gfs[:, 4:8])
    nc.vector.tensor_add(gfs[:, :2], gfs[:, :2], gfs[:, 2:4])
    acc0 = singles.tile([128, OUT_DIM, N_CAPS], bf16)
    nc.vector.tensor_add(acc0[:], gfs[:, 0], gfs[:, 1])
    for h in (0, 1):
        nc.sync.dma_start(cc_in[0][h][:], acc0[:, h * DH:(h + 1) * DH, :])
        allreduce(0, h)

    # ---- phase 2: routing, d-split pipelined around the ARs ----
    tmp = singles.tile([128, I1, DH, N_CAPS], bf16)
    b_log = singles.tile([128, I1, N_CAPS], f32)

    def squash_sqrt(ss_in, NP):
        # sqrt(ss+eps) via exp(0.5*ln(ss+eps)) on the ACT engine, so Ln/Exp
        # stay in one activation-table set; emitted BEFORE the bu-mul so
        # ACT streams while the DVE is busy on the big multiply
        t1 = small.tile([NP, N_CAPS], f32)
        nc.scalar.activation(
            out=t1[:], in_=ss_in[:], func=mybir.ActivationFunctionType.Ln,
            bias=eps_t[0:NP], scale=1.0,
        )
        tq = small.tile([NP, N_CAPS], f32)
        nc.scalar.activation(
            out=tq[:], in_=t1[:], func=mybir.ActivationFunctionType.Exp,
            bias=0.0, scale=0.5,
        )
        return tq

    def squash_fin(ss_in, tq, NP, inv):
        # scale = inv * ss/(1+ss)/sqrt(ss+eps)
        t2 = small.tile([NP, N_CAPS], f32)
        nc.vector.scalar_tensor_tensor(
            t2[:], ss_in[:], 1.0, tq[:],
            mybir.AluOpType.add, mybir.AluOpType.mult,
        )   # (1+ss)*sqrt(ss+eps)
        nc.vector.reciprocal(out=t2[:], in_=t2[:])
        t1b = small.tile([NP, N_CAPS], bf16)
        nc.vector.scalar_tensor_tensor(
            t1b[:], ss_in[:], inv, t2[:],
            mybir.AluOpType.mult, mybir.AluOpType.mult,
        )   # squash scale (*inv), bf16
        return t1b

    def bu_half(sh, h):
        # sum_{d in half} s*u_hat (squash scale deferred to after the fold:
        # b_log += sum_d out*u = scale[b,j] * sum_d s*u, algebraically equal)
        nc.vector.tensor_mul(
            tmp[:], u_hat[:, :, h * DH:(h + 1) * DH],
            sh.unsqueeze(1).broadcast_to([128, I1, DH, N_CAPS]),
        )
        w = DH
        while w > 2:
            nc.vector.tensor_add(
                tmp[:, :, :w // 2], tmp[:, :, :w // 2], tmp[:, :, w // 2:w]
            )
            w //= 2
        ph = small.tile([128, I1, N_CAPS], bf16, name=f"part{h}")
        nc.vector.tensor_add(ph[:], tmp[:, :, 0], tmp[:, :, 1])
        return ph

    for r in range(R):
        last = r == R - 1
        NP = B if last else 128
        # iteration 0's uniform c = 1/16 is folded into the squash math:
        # with s' = 16*s, ss = sum_d (s'/16)^2 and out = f(ss) * (s'/16)
        inv = 1.0 / N_CAPS if r == 0 else 1.0

        part = [None, None]
        s_h = [None, None]
        ss_h = [None, None]
        for h in (0, 1):
            # AR(r) half-h result -> SBUF on 4 parallel DMA queues; bu for
            # half 0 runs while half 1's AR is still in flight on the CC
            # stream. Duplicate into both partition halves (except the
            # last iter) so bu runs on 128 partitions.
            s_lo = small.tile([NP, DH, N_CAPS], bf16, name=f"slo{h}")
            s_hi = small.tile([NP, DH, N_CAPS], bf16, name=f"shi{h}")
            nc.sync.dma_start(s_lo[0:B], cc_out[r][h][0])
            nc.scalar.dma_start(s_hi[0:B], cc_out[r][h][1])
            if not last:
                nc.gpsimd.dma_start(s_lo[B:2 * B], cc_out[r][h][0])
                nc.sync.dma_start(s_hi[B:2 * B], cc_out[r][h][1])
            sh = small.tile([NP, DH, N_CAPS], bf16, name=f"sh{h}")
            nc.vector.tensor_add(sh[:], s_lo[:], s_hi[:])
            s_h[h] = sh
            # squash pieces: sq = (inv*s)^2, ss_h = sum_{d in half} sq
            sq = small.tile([NP, DH, N_CAPS], bf16, name=f"sq{h}")
            nc.vector.scalar_tensor_tensor(
                sq[:], sh[:], inv * inv, sh[:],
                mybir.AluOpType.mult, mybir.AluOpType.mult,
            )
            ssh = small.tile([NP, N_CAPS], f32, name=f"ssh{h}")
            with nc.allow_low_precision(
                reason="16-term bf16 square-sum into f32; 0.2% on scale"
            ):
                nc.vector.reduce_sum(
                    out=ssh[:], in_=sq.transpose([0, 2, 1]),
                    axis=mybir.AxisListType.X,
                )
            ss_h[h] = ssh
            if h == 0:
                if not last:
                    part[0] = bu_half(sh, 0)
                continue
            ss = small.tile([NP, N_CAPS], f32)
            nc.vector.tensor_add(ss[:], ss_h[0][:], ss_h[1][:])
            tq = squash_sqrt(ss, NP)
            if not last:
                part[1] = bu_half(sh, 1)
            t1b = squash_fin(ss, tq, NP, inv)

        if last:
            # out[b,j,d] = scale * s via a transposed-AP write
            out_t = small.tile([B, N_CAPS, OUT_DIM], f32)
            for h in (0, 1):
                nc.vector.tensor_mul(
                    out_t[:, :, h * DH:(h + 1) * DH].transpose([0, 2, 1]),
                    s_h[h][:],
                    t1b.unsqueeze(1).broadcast_to([B, DH, N_CAPS]),
                )
            nc.sync.dma_start(out[:], out_t[:])
            break

        # b_log update: b_log += scale * (part0 + part1)
        agr = small.tile([128, I1, N_CAPS], bf16)
        nc.vector.tensor_add(agr[:], part[0][:], part[1][:])
        tb = t1b.unsqueeze(1).broadcast_to([128, I1, N_CAPS])
        if r == 0:
            nc.vector.tensor_mul(b_log[:], agr[:], tb)
        else:
            tmul = small.tile([128, I1, N_CAPS], bf16)
            nc.vector.tensor_mul(tmul[:], agr[:], tb)
            nc.vector.tensor_add(b_log[:], b_log[:], tmul[:])

        # softmax over j: |b| stays < ~20, exp is fp32-safe without the
        # max-subtract. cexp/reciprocal in bf16 keep the DVE 2x mode.
        cexp = small.tile([128, I1, N_CAPS], bf16)
        nc.scalar.activation(
            out=cexp[:], in_=b_log[:],
            func=mybir.ActivationFunctionType.Exp,
        )
        csum = small.tile([128, I1], f32)
        with nc.allow_low_precision(
            reason="16-term bf16 sum into f32; 0.4% noise on c"
        ):
            nc.vector.reduce_sum(
                out=csum[:], in_=cexp[:], axis=mybir.AxisListType.X
            )
        csum_r = small.tile([128, I1], bf16)
        with nc.allow_low_precision(
            reason="bf16 softmax reciprocal; 0.4% noise on c"
        ):
            nc.vector.reciprocal(out=csum_r[:], in_=csum[:])
        c_t = small.tile([128, I1, N_CAPS], bf16)
        nc.vector.tensor_mul(
            c_t[:], cexp[:],
            csum_r.unsqueeze(2).broadcast_to([128, I1, N_CAPS]),
        )
        # s(r+1) partials per d-half; AR triggers as each half folds
        for h in (0, 1):
            nc.vector.tensor_mul(
                tmp[:], u_hat[:, :, h * DH:(h + 1) * DH],
                c_t.unsqueeze(2).broadcast_to([128, I1, DH, N_CAPS]),
            )
            w = I1
            while w > 2:
                nc.vector.tensor_add(
                    tmp[:, :w // 2], tmp[:, :w // 2], tmp[:, w // 2:w]
                )
                w //= 2
            shh = small.tile([128, DH, N_CAPS], bf16, name=f"shalf{h}")
            nc.vector.tensor_add(shh[:], tmp[:, 0], tmp[:, 1])
            nc.sync.dma_start(cc_in[r + 1][h][:], shh[:])
            allreduce(r + 1, h)

    ctx.close()


class _single_act_table:
    """Make every activation resolve to the one table set that covers
    Exp+Ln+Copy (natural_log_exp_and_others), so the kernel loads activation
    tables exactly once instead of thrashing Exp<->Ln sets (~1.3us per
    reload, on the critical path). Positional set ids are preserved, so the
    walrus side (which indexes the same act_info.json) stays consistent.
    Scoped: restores the original resolver on exit."""

    def __enter__(self):
        import concourse.bacc as bacc

        self._bacc = bacc
        self._orig = orig = bacc.get_activation_tables

        def patched(arch):
            tables = dict(orig(arch))
            keep = "natural_log_exp_and_others"
            if keep in tables:
                for k in tables:
                    if k != keep:
                        tables[k] = set()
            return tables

        bacc.get_activation_tables = patched
        return self

    def __exit__(self, *exc):
        self._bacc.get_activation_tables = self._orig
        return False


def _build(num_routing):
    import concourse.bacc as bacc
    import concourse.tile as tile
    from concourse import mybir

    nc = bacc.Bacc(
        "TRN2", target_bir_lowering=False, debug=False, num_devices=N_CORES,
        dynamic_dma_scratch_size=512,
    )
    f32 = mybir.dt.float32
    bf16 = mybir.dt.bfloat16
    xT = nc.dram_tensor("xT", [IN_DIM, I_LOC, B], bf16, kind="ExternalInput")
    wT = nc.dram_tensor(
        "wT", [NPAIR, IN_DIM, 2, GRP, OUT_DIM, N_CAPS], bf16,
        kind="ExternalInput",
    )
    out = nc.dram_tensor(
        "out", [B, N_CAPS, OUT_DIM], f32, kind="ExternalOutput"
    )
    with tile.TileContext(nc) as tc:
        _emit(tc, xT, wT, out, num_routing)
    with _single_act_table():
        nc.compile()
    return nc


def kernel(inputs, W, num_routing):
    import ml_dtypes

    from concourse.bass_utils import run_bass_kernel_spmd

    R = int(num_routing)
    assert R >= 1
    if R not in _cache:
        _cache[R] = _build(R)
    nc = _cache[R]

    bf = ml_dtypes.bfloat16
    inputs = np.ascontiguousarray(np.asarray(inputs, dtype=np.float32))
    W = np.asarray(W, dtype=np.float32)

    in_maps = []
    for c in range(N_CORES):
        lo, hi = c * I_LOC, (c + 1) * I_LOC
        xT_c = np.ascontiguousarray(
            inputs[:, lo:hi, :].transpose(2, 1, 0).astype(bf)
        )
        # [i,j,k,d] -> pair-blocked [p, k, g2, t, d, j]: each 2-group DMA is
        # one contiguous 1MB block with 8KB contiguous per partition line
        wT_c = np.ascontiguousarray(
            W[lo:hi]
            .reshape(NPAIR, 2, GRP, N_CAPS, IN_DIM, OUT_DIM)
            .transpose(0, 4, 1, 2, 5, 3)
            .astype(bf)
        )
        in_maps.append({"xT": xT_c, "wT": wT_c})

    kwargs = {}
    if TRACE:
        kwargs["trace"] = True
        if TRACE_DIR:
            kwargs["tmpdir"] = TRACE_DIR
    res = None
    for attempt in range(3):
        try:
            res = run_bass_kernel_spmd(
                nc, in_maps, core_ids=list(range(N_CORES)), **kwargs
            )
            break
        except Exception:
            if attempt == 2:
                raise
            import time
            time.sleep(5)
    if TRACE:
        kernel.last_exec_time_ns = res.exec_time_ns
        kernel.last_results = res
    return np.asarray(res.results[0]["out"], dtype=np.float32)


# revision 27
# speedup vs baseline: 1.3815x; 1.3815x over previous
"""CapsuleLayer (dynamic routing) Trainium2 kernel, SPMD over 8 NeuronCores.

Sharding: input-capsule axis (IN_CAPS=512 -> 64 per core). W and u_hat are
i-sharded; the bij,bijd->bjd contraction is completed with AllReduces of
bf16 s-partials once per routing iteration.

Per-core layout (i_local = i2*32 + i1, i2 in {0,1}):
  u_hat SBUF [p=(i2*64+b), (d, i1, j)] bf16 -- 128 partitions x 16384
  b/c logits [p, (i1, j)], s partials [p, (d, j)].

v2 structure (over the 245us baseline):
- No warmup AllReduce. Instead, the r=0 s-partial (c uniform -> s0 =
  (1/16) sum_i u_hat, the 1/16 folded into the squash scale) is built
  incrementally DURING phase 1: each 4-i group is folded 4->1 as its
  PSUM cast lands, staged into gfs[128, 8, (d,j)], and tree-folded 8->1
  right after the last cast. Its AllReduce triggers at ~46us, so the AR
  starts the moment the NRT collective-init barrier (48-145us, run
  variance) ends, instead of queueing behind a warmup AR + DVE fold
  (~36us of serialized warmup+AR0 on the baseline critical path).
- Every s AllReduce is split into two d-halves (64 KB bf16 each) on the
  single CC stream. Downstream work is d-decomposable: the agreement
  update b_log += sum_d out*u_hat = scale[b,j] * sum_d s*u_hat (squash
  scale applied AFTER the d-fold, algebraically identical), so the
  bu-mul+fold for half A runs while half B's AR is still in flight.
  Per-iteration AR exposure drops from ~12.5us + full serial DVE to
  mostly-hidden.
- cc buffers are dedicated (bufs = exact tile count) so no DRAM-pool
  aliasing creates false WAR semaphore deps on the AR triggers.
- sqrt(ss+eps) = exp(0.5*ln(ss+eps)) with activation tables pinned to
  the one set containing Ln+Exp+Copy (no per-iteration table reloads).

Measured notes (from traces): DVE tensor_tensor bf16 runs 2x only with
step-1 innermost and full-width ops; broadcast over a non-innermost dim
keeps 2x. AR latency is fixed ~11-13us nearly independent of payload
(128B vs 128KB), so splitting costs stream time but buys overlap. The
NRT barrier (collective stack init) ends 48-145us into the kernel and
gates the first AR; nothing in-kernel controls it. GPSIMD ~1.9ns/elem
flat, ~4.7 sliced/broadcast. remote_dma hangs this runtime (axon
fake_nrt), so the collective stack is unavoidable.

Phase 1 (per i): u_hat_i[b, dj] = xT_i.T @ W_i on the PE (K=128, M=64,
N=512), all in bf16, W streamed in 1 MB pair-blocked chunks.
"""

import numpy as np

N_CORES = 8
B = 64
IN_CAPS = 512
IN_DIM = 128
N_CAPS = 16
OUT_DIM = 32
I_LOC = IN_CAPS // N_CORES          # 64 input capsules per core
I1 = 32                             # i_local = i2*32 + i1
DH = OUT_DIM // 2                   # 16, d-half for split ARs
EPS = 1e-7
GRP = 4                             # i's per PSUM group
NGRP = I_LOC // GRP                 # 16
NPAIR = NGRP // 2                   # 8 (two groups per W DMA)

# Toggled by test.py for profiling runs.
TRACE = False
TRACE_DIR = None

_cache = {}


def _emit(tc, xT, wT, out, num_routing):
    from contextlib import ExitStack

    from concourse import mybir

    nc = tc.nc
    f32 = mybir.dt.float32
    bf16 = mybir.dt.bfloat16
    R = num_routing
    ctx = ExitStack()
    singles = ctx.enter_context(tc.tile_pool(name="singles", bufs=1))
    # all 16 W groups resident: W DMAs never wait on a slot's prior reader,
    # so both DMA queues can post their whole half of the stream up front
    wpool = ctx.enter_context(tc.tile_pool(name="wpool", bufs=NGRP))
    pspool = ctx.enter_context(tc.tile_pool(name="pspool", bufs=2, space="PSUM"))
    small = ctx.enter_context(tc.tile_pool(name="small", bufs=2))
    gpool = ctx.enter_context(tc.tile_pool(name="gpool", bufs=2))
    # one slot per cc buffer: zero reuse -> no false WAR deps on triggers
    ccpool = ctx.enter_context(tc.tile_pool(name="ccpool", bufs=4 * R, space="DRAM"))

    # All ARs are d-split 64KB halves: the first-AR premium scales with
    # payload (64KB first AR ~15us vs 128KB first AR ~30us), and the
    # split lets bu-mul(half A) start while half B's AR is in flight.
    cc_in = [[None, None] for _ in range(R)]
    cc_out = [[None, None] for _ in range(R)]
    for r in range(R):
        for h in (0, 1):
            cc_in[r][h] = ccpool.tile(
                [2, B, DH, N_CAPS], bf16, name=f"ccin{r}{h}"
            )
            cc_out[r][h] = ccpool.tile(
                [2, B, DH, N_CAPS], bf16, name=f"ccout{r}{h}"
            )

    def allreduce(r, h):
        nc.gpsimd.collective_compute(
            "AllReduce",
            mybir.AluOpType.add,
            replica_groups=[list(range(N_CORES))],
            ins=[cc_in[r][h].opt()],
            outs=[cc_out[r][h].opt()],
        )

    # ---- phase 1: u_hat = einsum over k, per local capsule i; the r=0
    # s-partial sum_i u_hat accumulates alongside ----
    xsb = singles.tile([IN_DIM, I_LOC, B], bf16)         # [k, i, b]
    # free order (i1, d, j): the PSUM->SBUF cast is a straight contiguous
    # copy (PSUM groups are [b, i-in-group, d, j]), and every routing op
    # stays in the DVE bf16 2x mode: smul broadcasts c over middle d, bu
    # broadcasts s over outermost i1, folds slice outermost i1 / middle d
    u_hat = singles.tile([128, I1, OUT_DIM, N_CAPS], bf16)  # [(i2,b), i1, d, j]
    gfs = singles.tile([128, NPAIR, OUT_DIM, N_CAPS], bf16)  # group partials
    eps_t = singles.tile([128, 1], f32)
    nc.vector.memset(eps_t[:], EPS)

    XCH = I_LOC // 4
    for g in range(NGRP):
        # split the 8MB W stream across BOTH HWDGE queues (sync+scalar) in
        # per-group 512KB blocks: one queue tops out ~220GB/s; two reach
        # the HBM limit, and small blocks start the PE ~4us earlier. x
        # chunks lead their queue so the first matmul's operands land
        # first.
        dq = nc.sync if g % 2 == 0 else nc.scalar
        if g < 4:
            dq.dma_start(
                xsb[:, g * XCH:(g + 1) * XCH, :],
                xT[:, g * XCH:(g + 1) * XCH, :],
            )
        wtile = wpool.tile([IN_DIM, GRP, OUT_DIM, N_CAPS], bf16)
        dq.dma_start(wtile[:], wT[g])
        i2 = (g * GRP) // I1
        i1g = (g * GRP) % I1
        gh = (g % NPAIR)  # staging slot within this i2-half
        ps = pspool.tile([128, GRP, OUT_DIM, N_CAPS], f32)
        for t in range(GRP):
            i = g * GRP + t
            nc.tensor.matmul(
                ps[i2 * B:(i2 + 1) * B, t], xsb[:, i, :], wtile[:, t],
                start=True, stop=True,
            )
        # copy+cast PSUM f32 -> SBUF bf16: the ~2us f32-src casts are the
        # phase-1 engine floor (~33us total), split 10 ACT / 6 DVE so both
        # cast chains drain together (GPSIMD cannot read PSUM)
        if g % 8 < 5:
            nc.scalar.copy(
                out=u_hat[i2 * B:(i2 + 1) * B, i1g:i1g + GRP],
                in_=ps[i2 * B:(i2 + 1) * B],
            )
        else:
            nc.vector.tensor_copy(
                out=u_hat[i2 * B:(i2 + 1) * B, i1g:i1g + GRP],
                in_=ps[i2 * B:(i2 + 1) * B],
            )
        # incremental r0 fold: this group's 4 i's -> gfs slot; half the
        # folds run on the otherwise-idle GPSIMD (flat contiguous APs)
        ug = u_hat[i2 * B:(i2 + 1) * B, i1g:i1g + GRP]
        if g % 2 == 0:
            gtmp = gpool.tile([B, 2, OUT_DIM, N_CAPS], bf16, name="gtmpg")
            nc.gpsimd.tensor_add(gtmp[:], ug[:, 0:2], ug[:, 2:4])
            nc.gpsimd.tensor_add(
                gfs[i2 * B:(i2 + 1) * B, gh], gtmp[:, 0], gtmp[:, 1]
            )
        else:
            gtmp = gpool.tile([B, 2, OUT_DIM, N_CAPS], bf16, name="gtmpv")
            nc.vector.tensor_add(gtmp[:], ug[:, 0:2], ug[:, 2:4])
            nc.vector.tensor_add(
                gfs[i2 * B:(i2 + 1) * B, gh], gtmp[:, 0], gtmp[:, 1]
            )
    # tree-fold the 8 group partials per half -> acc0 = sum_i u_hat
    nc.vector.tensor_add(gfs[:, :4], gfs[:, :4], gfs[:, 4:8])
    nc.vector.tensor_add(gfs[:, :2], gfs[:, :2], gfs[:, 2:4])
    acc0 = singles.tile([128, OUT_DIM, N_CAPS], bf16)
    nc.vector.tensor_add(acc0[:], gfs[:, 0], gfs[:, 1])
    for h in (0, 1):
        nc.sync.dma_start(cc_in[0][h][:], acc0[:, h * DH:(h + 1) * DH, :])
        allreduce(0, h)

    # ---- phase 2: routing, d-split pipelined around the ARs ----
    tmp = singles.tile([128, I1, DH, N_CAPS], bf16)
    b_log = singles.tile([128, I1, N_CAPS], f32)

    def squash_sqrt(ss_in, NP):
        # sqrt(ss+eps) via exp(0.5*ln(ss+eps)) on the ACT engine, so Ln/Exp
        # stay in one activation-table set; emitted BEFORE the bu-mul so
        # ACT streams while the DVE is busy on the big multiply
        t1 = small.tile([NP, N_CAPS], f32)
        nc.scalar.activation(
            out=t1[:], in_=ss_in[:], func=mybir.ActivationFunctionType.Ln,
            bias=eps_t[0:NP], scale=1.0,
        )
        tq = small.tile([NP, N_CAPS], f32)
        nc.scalar.activation(
            out=tq[:], in_=t1[:], func=mybir.ActivationFunctionType.Exp,
            bias=0.0, scale=0.5,
        )
        return tq

    def squash_fin(ss_in, tq, NP, inv):
        # scale = inv * ss/(1+ss)/sqrt(ss+eps)
        t2 = small.tile([NP, N_CAPS], f32)
        nc.vector.scalar_tensor_tensor(
            t2[:], ss_in[:], 1.0, tq[:],
            mybir.AluOpType.add, mybir.AluOpType.mult,
        )   # (1+ss)*sqrt(ss+eps)
        nc.vector.reciprocal(out=t2[:], in_=t2[:])
        t1b = small.tile([NP, N_CAPS], bf16)
        nc.vector.scalar_tensor_tensor(
            t1b[:], ss_in[:], inv, t2[:],
            mybir.AluOpType.mult, mybir.AluOpType.mult,
        )   # squash scale (*inv), bf16
        return t1b

    def bu_half(sh, h):
        # sum_{d in half} s*u_hat (squash scale deferred to after the fold:
        # b_log += sum_d out*u = scale[b,j] * sum_d s*u, algebraically equal)
        nc.vector.tensor_mul(
            tmp[:], u_hat[:, :, h * DH:(h + 1) * DH],
            sh.unsqueeze(1).broadcast_to([128, I1, DH, N_CAPS]),
        )
        w = DH
        while w > 2:
            nc.vector.tensor_add(
                tmp[:, :, :w // 2], tmp[:, :, :w // 2], tmp[:, :, w // 2:w]
            )
            w //= 2
        ph = small.tile([128, I1, N_CAPS], bf16, name=f"part{h}")
        nc.vector.tensor_add(ph[:], tmp[:, :, 0], tmp[:, :, 1])
        return ph

    for r in range(R):
        last = r == R - 1
        NP = B if last else 128
        # iteration 0's uniform c = 1/16 is folded into the squash math:
        # with s' = 16*s, ss = sum_d (s'/16)^2 and out = f(ss) * (s'/16)
        inv = 1.0 / N_CAPS if r == 0 else 1.0

        part = [None, None]
        s_h = [None, None]
        ss_h = [None, None]
        for h in (0, 1):
            # AR(r) half-h result -> SBUF on 4 parallel DMA queues; bu for
            # half 0 runs while half 1's AR is still in flight on the CC
            # stream. Duplicate into both partition halves (except the
            # last iter) so bu runs on 128 partitions.
            s_lo = small.tile([NP, DH, N_CAPS], bf16, name=f"slo{h}")
            s_hi = small.tile([NP, DH, N_CAPS], bf16, name=f"shi{h}")
            nc.sync.dma_start(s_lo[0:B], cc_out[r][h][0])
            nc.scalar.dma_start(s_hi[0:B], cc_out[r][h][1])
            if not last:
                nc.gpsimd.dma_start(s_lo[B:2 * B], cc_out[r][h][0])
                nc.sync.dma_start(s_hi[B:2 * B], cc_out[r][h][1])
            sh = small.tile([NP, DH, N_CAPS], bf16, name=f"sh{h}")
            nc.vector.tensor_add(sh[:], s_lo[:], s_hi[:])
            s_h[h] = sh
            # squash pieces: sq = (inv*s)^2, ss_h = sum_{d in half} sq
            sq = small.tile([NP, DH, N_CAPS], bf16, name=f"sq{h}")
            nc.vector.scalar_tensor_tensor(
                sq[:], sh[:], inv * inv, sh[:],
                mybir.AluOpType.mult, mybir.AluOpType.mult,
            )
            ssh = small.tile([NP, N_CAPS], f32, name=f"ssh{h}")
            with nc.allow_low_precision(
                reason="16-term bf16 square-sum into f32; 0.2% on scale"
            ):
                nc.vector.reduce_sum(
                    out=ssh[:], in_=sq.transpose([0, 2, 1]),
                    axis=mybir.AxisListType.X,
                )
            ss_h[h] = ssh
            if h == 0:
                if not last:
                    part[0] = bu_half(sh, 0)
                continue
            ss = small.tile([NP, N_CAPS], f32)
            nc.vector.tensor_add(ss[:], ss_h[0][:], ss_h[1][:])
            tq = squash_sqrt(ss, NP)
            if not last:
                part[1] = bu_half(sh, 1)
            t1b = squash_fin(ss, tq, NP, inv)

        if last:
            # out[b,j,d] = scale * s via a transposed-AP write
            out_t = small.tile([B, N_CAPS, OUT_DIM], f32)
            for h in (0, 1):
                nc.vector.tensor_mul(
                    out_t[:, :, h * DH:(h + 1) * DH].transpose([0, 2, 1]),
                    s_h[h][:],
                    t1b.unsqueeze(1).broadcast_to([B, DH, N_CAPS]),
                )
            nc.sync.dma_start(out[:], out_t[:])
            break

        # b_log update: b_log += scale * (part0 + part1)
        agr = small.tile([128, I1, N_CAPS], bf16)
        nc.vector.tensor_add(agr[:], part[0][:], part[1][:])
        tb = t1b.unsqueeze(1).broadcast_to([128, I1, N_CAPS])
        if r == 0:
            nc.vector.tensor_mul(b_log[:], agr[:], tb)
        else:
            tmul = small.tile([128, I1, N_CAPS], bf16)
            nc.vector.tensor_mul(tmul[:], agr[:], tb)
            nc.vector.tensor_add(b_log[:], b_log[:], tmul[:])

        # softmax over j: |b| stays < ~20, exp is fp32-safe without the
        # max-subtract. cexp/reciprocal in bf16 keep the DVE 2x mode.
        cexp = small.tile([128, I1, N_CAPS], bf16)
        nc.scalar.activation(
            out=cexp[:], in_=b_log[:],
            func=mybir.ActivationFunctionType.Exp,
        )
        csum = small.tile([128, I1], f32)
        with nc.allow_low_precision(
            reason="16-term bf16 sum into f32; 0.4% noise on c"
        ):
            nc.vector.reduce_sum(
                out=csum[:], in_=cexp[:], axis=mybir.AxisListType.X
            )
        csum_r = small.tile([128, I1], bf16)
        with nc.allow_low_precision(
            reason="bf16 softmax reciprocal; 0.4% noise on c"
        ):
            nc.vector.reciprocal(out=csum_r[:], in_=csum[:])
        c_t = small.tile([128, I1, N_CAPS], bf16)
        nc.vector.tensor_mul(
            c_t[:], cexp[:],
            csum_r.unsqueeze(2).broadcast_to([128, I1, N_CAPS]),
        )
        # s(r+1) partials per d-half; AR triggers as each half folds
        for h in (0, 1):
            nc.vector.tensor_mul(
                tmp[:], u_hat[:, :, h * DH:(h + 1) * DH],
                c_t.unsqueeze(2).broadcast_to([128, I1, DH, N_CAPS]),
            )
            w = I1
            while w > 2:
                nc.vector.tensor_add(
                    tmp[:, :w // 2], tmp[:, :w // 2], tmp[:, w // 2:w]
                )
                w //= 2
            shh = small.tile([128, DH, N_CAPS], bf16, name=f"shalf{h}")
            nc.vector.tensor_add(shh[:], tmp[:, 0], tmp[:, 1])
            nc.sync.dma_start(cc_in[r + 1][h][:], shh[:])
            allreduce(r + 1, h)

    ctx.close()


class _single_act_table:
    """Make every activation resolve to the one table set that covers
    Exp+Ln+Copy (natural_log_exp_and_others), so the kernel loads activation
    tables exactly once instead of thrashing Exp<->Ln sets (~1.3us per
    reload, on the critical path). Positional set ids are preserved, so the
    walrus side (which indexes the same act_info.json) stays consistent.
    Scoped: restores the original resolver on exit."""

    def __enter__(self):
        import concourse.bacc as bacc

        self._bacc = bacc
        self._orig = orig = bacc.get_activation_tables

        def patched(arch):
            tables = dict(orig(arch))
            keep = "natural_log_exp_and_others"
            if keep in tables:
                for k in tables:
                    if k != keep:
                        tables[k] = set()
            return tables

        bacc.get_activation_tables = patched
        return self

    def __exit__(self, *exc):
        self._bacc.get_activation_tables = self._orig
        return False


def _build(num_routing):
    import concourse.bacc as bacc
    import concourse.tile as tile
    from concourse import mybir

    nc = bacc.Bacc(
        "TRN2", target_bir_lowering=False, debug=False, num_devices=N_CORES,
        dynamic_dma_scratch_size=512,
    )
    f32 = mybir.dt.float32
    bf16 = mybir.dt.bfloat16
    xT = nc.dram_tensor("xT", [IN_DIM, I_LOC, B], bf16, kind="ExternalInput")
    wT = nc.dram_tensor(
        "wT", [NGRP, IN_DIM, GRP, OUT_DIM, N_CAPS], bf16,
        kind="ExternalInput",
    )
    out = nc.dram_tensor(
        "out", [B, N_CAPS, OUT_DIM], f32, kind="ExternalOutput"
    )
    with tile.TileContext(nc) as tc:
        _emit(tc, xT, wT, out, num_routing)
    with _single_act_table():
        nc.compile()
    return nc


def kernel(inputs, W, num_routing):
    import ml_dtypes

    from concourse.bass_utils import run_bass_kernel_spmd

    R = int(num_routing)
    assert R >= 1
    if R not in _cache:
        _cache[R] = _build(R)
    nc = _cache[R]

    bf = ml_dtypes.bfloat16
    inputs = np.ascontiguousarray(np.asarray(inputs, dtype=np.float32))
    W = np.asarray(W, dtype=np.float32)

    in_maps = []
    for c in range(N_CORES):
        lo, hi = c * I_LOC, (c + 1) * I_LOC
        xT_c = np.ascontiguousarray(
            inputs[:, lo:hi, :].transpose(2, 1, 0).astype(bf)
        )
        # [i,j,k,d] -> group-blocked [g, k, t, d, j]: each 4-i group DMA is
        # one contiguous 512KB block with 4KB contiguous per partition line
        wT_c = np.ascontiguousarray(
            W[lo:hi]
            .reshape(NGRP, GRP, N_CAPS, IN_DIM, OUT_DIM)
            .transpose(0, 3, 1, 4, 2)
            .astype(bf)
        )
        in_maps.append({"xT": xT_c, "wT": wT_c})

    kwargs = {}
    if TRACE:
        kwargs["trace"] = True
        if TRACE_DIR:
            kwargs["tmpdir"] = TRACE_DIR
    res = None
    for attempt in range(3):
        try:
            res = run_bass_kernel_spmd(
                nc, in_maps, core_ids=list(range(N_CORES)), **kwargs
            )
            break
        except Exception:
            if attempt == 2:
                raise
            import time
            time.sleep(5)
    if TRACE:
        kernel.last_exec_time_ns = res.exec_time_ns
        kernel.last_results = res
    return np.asarray(res.results[0]["out"], dtype=np.float32)
